# revision 1
# baseline (speedup 1.0000x reference)
"""JKNet-Maxpool GNN kernel for 8 Trainium2 NeuronCores.

Strategy (graph/data parallel, dense-adjacency aggregation):
  - Shard dst nodes 8 ways (1250/core, padded to 1280 = 10 tiles of 128).
  - segment_sum over edges == A @ m with A[dst, src] the edge-count matrix.
    A entries are small ints -> exact in bf16.  Aggregation runs on the PE as
    dense matmuls: stationary = m chunks [128 src, 128 feat] (bf16), moving =
    A^T chunks [128 src, <=512 dst] (bf16, streamed from HBM), accumulated in
    fp32 PSUM over all 80 src tiles.
  - Transposed dataflow: activations live as x^T [feat_part, node_free], so
    the per-layer GEMM (fp32 for accuracy) uses x^T chunks as the stationary
    operand and W as the moving operand with zero transposes anywhere.
  - Per layer: local GEMM -> cast bf16 -> AllGather m across the 8 cores ->
    dense aggregation -> ReLU+bias -> running JK max.
  - Final GCN layer uses the normalization-weighted adjacency Aw (built on
    host, includes the self-loop 1/deg diagonal), then log_softmax.
"""

import numpy as np
import ml_dtypes

import concourse.bass as bass
import concourse.bacc as bacc
import concourse.mybir as mybir
import concourse.tile as tile
from concourse.bass_utils import run_bass_kernel_spmd
from concourse.masks import make_identity

BF16 = mybir.dt.bfloat16
F32 = mybir.dt.float32
AF = mybir.ActivationFunctionType
ALU = mybir.AluOpType
AX = mybir.AxisListType

# ---------------------------------------------------------------- config
class Cfg:
    def __init__(self, n_nodes, in_feats, units, out_feats, n_layers, n_cores=8):
        self.P = 128
        self.C = n_cores
        self.N = n_nodes
        self.IN = in_feats            # multiple of 128
        self.U = units                # multiple of 128
        self.OUTP = 128               # padded out feats (real out <= 128)
        self.L = n_layers             # hidden GCN layers
        nloc_real = (n_nodes + n_cores - 1) // n_cores
        self.NLOC_REAL = nloc_real
        self.NT_LOC = (nloc_real + 127) // 128
        self.NLOC = self.NT_LOC * 128            # padded local nodes
        self.KT = self.C * self.NT_LOC           # src tiles over padded space
        self.NFULL = self.KT * 128
        self.KT_IN = in_feats // 128
        self.KT_U = units // 128
        # moving-dim slices for aggregation matmuls (<=512 each)
        self.SLICES = []
        off = 0
        while off < self.NLOC:
            w = min(512, self.NLOC - off)
            self.SLICES.append((off, w))
            off += w


REAL = Cfg(n_nodes=10000, in_feats=512, units=256, out_feats=64, n_layers=6)
OUT_REAL = 64


# ---------------------------------------------------------------- program

DMA_ENGINE = "gpsimd"  # "sync" (HWDGE) or "gpsimd" (SWDGE)


def _dma(nc):
    return (nc.gpsimd if DMA_ENGINE == "gpsimd" else nc.sync).dma_start

def build_nc(cfg: Cfg) -> bass.Bass:
    nc = bacc.Bacc("TRN2", target_bir_lowering=False, num_devices=cfg.C)
    P, L = cfg.P, cfg.L

    # ---- dram I/O (per-core contents supplied via in_maps)
    hT_d = nc.dram_tensor("hT", [cfg.KT_IN, P, cfg.NLOC], F32, kind="ExternalInput")
    AT_d = nc.dram_tensor("AT", [cfg.KT, P, cfg.NLOC], BF16, kind="ExternalInput")
    AwT_d = nc.dram_tensor("AwT", [cfg.KT, P, cfg.NLOC], BF16, kind="ExternalInput")
    W0_d = nc.dram_tensor("W0", [cfg.KT_IN, P, cfg.U], F32, kind="ExternalInput")
    Wh_d = nc.dram_tensor("Wh", [L - 1, cfg.KT_U, P, cfg.U], F32, kind="ExternalInput")
    Wo_d = nc.dram_tensor("Wo", [cfg.KT_U, P, cfg.OUTP], F32, kind="ExternalInput")
    # packed per-partition biases: col l*2+ft = bias for layer l feat tile ft,
    # col 2L = final bias (bo padded)
    nb = 2 * L + 1 if cfg.KT_U == 2 else cfg.KT_U * L + 1
    nb = cfg.KT_U * L + 1
    bias_d = nc.dram_tensor("biases", [P, nb], F32, kind="ExternalInput")
    out_d = nc.dram_tensor("out", [cfg.NLOC, OUT_REAL], F32, kind="ExternalOutput")

    with tile.TileContext(nc) as tc:
        with (
            tc.tile_pool(name="const", bufs=1) as const_p,
            tc.tile_pool(name="wpool", bufs=1) as w_p,
            tc.tile_pool(name="xT", bufs=cfg.KT_IN + cfg.KT_U + 2) as x_p,
            tc.tile_pool(name="jk", bufs=1) as jk_p,
            tc.tile_pool(name="mfull", bufs=cfg.KT) as mf_p,
            tc.tile_pool(name="at", bufs=4) as at_p,
            tc.tile_pool(name="mloc", bufs=4) as ml_p,
            tc.tile_pool(name="small", bufs=6) as sm_p,
            tc.tile_pool(name="psmm", bufs=2, space="PSUM") as psmm_p,
            tc.tile_pool(name="psagg", bufs=2, space="PSUM") as psagg_p,
            tc.tile_pool(name="dram", bufs=1, space="DRAM") as dram_p,
        ):
            # ---- constants
            biases = const_p.tile([P, nb], F32, name="biases_sb")
            _dma(nc)(out=biases[:], in_=bias_d[:])
            ident = const_p.tile([P, P], F32, name="ident")
            make_identity(nc, ident[:])

            # ---- weights resident in SBUF
            w0_sb = []
            for k in range(cfg.KT_IN):
                t = w_p.tile([P, cfg.U], F32, name=f"w0_{k}")
                _dma(nc)(out=t[:], in_=W0_d[k])
                w0_sb.append(t)
            wh_sb = []
            for l in range(L - 1):
                row = []
                for k in range(cfg.KT_U):
                    t = w_p.tile([P, cfg.U], F32, name=f"wh_{l}_{k}")
                    _dma(nc)(out=t[:], in_=Wh_d[l, k])
                    row.append(t)
                wh_sb.append(row)
            wo_sb = []
            for k in range(cfg.KT_U):
                t = w_p.tile([P, cfg.OUTP], F32, name=f"wo_{k}")
                _dma(nc)(out=t[:], in_=Wo_d[k])
                wo_sb.append(t)

            # ---- x^T tiles (layer 0 = h^T)
            xT = []
            for k in range(cfg.KT_IN):
                t = x_p.tile([P, cfg.NLOC], F32, tag="xT", name=f"xt0_{k}")
                _dma(nc)(out=t[:], in_=hT_d[k])
                xT.append(t)

            # ---- JK running max tiles
            jk = [
                jk_p.tile([P, cfg.NLOC], F32, name=f"jk_{ft}")
                for ft in range(cfg.KT_U)
            ]

            # ---- collective bounce buffers
            m_loc_d = dram_p.tile([cfg.C, cfg.NT_LOC, P, cfg.U], BF16,
                                  name="m_loc_d")
            m_full_ds = [
                dram_p.tile([cfg.KT, P, cfg.U], BF16, name=f"m_full_d{l}",
                            addr_space="Shared")
                for l in range(L)
            ]
            mo_loc_d = dram_p.tile([cfg.C, cfg.NT_LOC, P, cfg.OUTP], BF16,
                                   name="mo_loc_d")
            mo_full_d = dram_p.tile([cfg.KT, P, cfg.OUTP], BF16,
                                    name="mo_full_d", addr_space="Shared")

            def gemm_allgather(xT_tiles, w_tiles, width, loc_d, full_d, lname):
                """m_loc = x_loc @ W (fp32), cast bf16, all-gather to SBUF tiles."""
                kt = len(xT_tiles)
                for nt in range(cfg.NT_LOC):
                    ps = psmm_p.tile([P, width], F32, tag="mm",
                                     name=f"ps_{lname}_{nt}")
                    for k in range(kt):
                        nc.tensor.matmul(
                            ps[:],
                            lhsT=xT_tiles[k][:, nt * P:(nt + 1) * P],
                            rhs=w_tiles[k][:],
                            start=(k == 0),
                            stop=(k == kt - 1),
                        )
                    mt = ml_p.tile([P, width], BF16, tag="mloc",
                                   name=f"m_{lname}_{nt}")
                    nc.vector.tensor_copy(out=mt[:], in_=ps[:])
                    # every core writes its shard into slot 0 of loc_d; the
                    # AllGather concatenates shards in replica order.
                    _dma(nc)(out=loc_d[0, nt], in_=mt[:])
                nc.gpsimd.collective_compute(
                    "AllGather",
                    ALU.bypass,
                    replica_groups=[list(range(cfg.C))],
                    ins=[loc_d[0].opt()],
                    outs=[full_d.opt()],
                )
                full_sb = []
                for k in range(cfg.KT):
                    t = mf_p.tile([P, width], BF16, tag="mfull",
                                  name=f"mf_{lname}_{k}")
                    _dma(nc)(out=t[:], in_=full_d[k])
                    full_sb.append(t)
                return full_sb

            def aggregate(full_sb, adjT_d, width, lname):
                """agg^T[feat, dst] += m_chunk.T @ A^T chunk, fp32 psum."""
                nft = width // P
                ps_list = [
                    psagg_p.tile([P, cfg.NLOC], F32, tag="agg",
                                 name=f"agg_{lname}_{ft}")
                    for ft in range(nft)
                ]
                for k in range(cfg.KT):
                    at = at_p.tile([P, cfg.NLOC], BF16, tag="at",
                                   name=f"at_{lname}_{k}")
                    _dma(nc)(out=at[:], in_=adjT_d[k])
                    for ft in range(nft):
                        for off, w in cfg.SLICES:
                            nc.tensor.matmul(
                                ps_list[ft][:, off:off + w],
                                lhsT=full_sb[k][:, ft * P:(ft + 1) * P],
                                rhs=at[:, off:off + w],
                                start=(k == 0),
                                stop=(k == cfg.KT - 1),
                            )
                return ps_list

            # ================= hidden layers =================
            for l in range(L):
                xt_in = xT
                w_tiles = w0_sb if l == 0 else wh_sb[l - 1]
                m_sb = gemm_allgather(xt_in, w_tiles, cfg.U,
                                      m_loc_d, m_full_ds[l], f"l{l}")
                ps_list = aggregate(m_sb, AT_d, cfg.U, f"l{l}")
                xT = []
                for ft in range(cfg.KT_U):
                    xt_new = x_p.tile([P, cfg.NLOC], F32, tag="xT",
                                      name=f"xt{l + 1}_{ft}")
                    nc.scalar.activation(
                        xt_new[:], ps_list[ft][:], AF.Relu,
                        bias=biases[:, cfg.KT_U * l + ft:cfg.KT_U * l + ft + 1],
                    )
                    xT.append(xt_new)
                    if l == 0:
                        nc.vector.tensor_copy(out=jk[ft][:], in_=xt_new[:])
                    else:
                        nc.vector.tensor_tensor(
                            out=jk[ft][:], in0=jk[ft][:], in1=xt_new[:],
                            op=ALU.max,
                        )

            # ================= final layer =================
            mo_sb = gemm_allgather(jk, wo_sb, cfg.OUTP,
                                   mo_loc_d, mo_full_d, "fin")
            ps_fin = aggregate(mo_sb, AwT_d, cfg.OUTP, "fin")[0]
            aggF = x_p.tile([P, cfg.NLOC], F32, tag="xT", name="aggF")
            nc.scalar.activation(
                aggF[:], ps_fin[:], AF.Identity,
                bias=biases[:, cfg.KT_U * L:cfg.KT_U * L + 1],
            )
            for nt in range(cfg.NT_LOC):
                ps_t = psmm_p.tile([P, P], F32, tag="mm", name=f"pst_{nt}")
                nc.tensor.transpose(
                    out=ps_t[:], in_=aggF[:, nt * P:(nt + 1) * P],
                    identity=ident[:],
                )
                z = ps_t[:, 0:OUT_REAL]
                rmax = sm_p.tile([P, 1], F32, tag="r1", name=f"rmax_{nt}")
                nc.vector.reduce_max(rmax[:], z, axis=AX.X)
                z2 = sm_p.tile([P, OUT_REAL], F32, tag="z2", name=f"z2_{nt}")
                nc.vector.tensor_scalar_sub(z2[:], z, rmax[:])
                ez = sm_p.tile([P, OUT_REAL], F32, tag="ez", name=f"ez_{nt}")
                nc.scalar.activation(ez[:], z2[:], AF.Exp)
                ssum = sm_p.tile([P, 1], F32, tag="r2", name=f"ssum_{nt}")
                nc.vector.reduce_sum(ssum[:], ez[:], axis=AX.X)
                lsum = sm_p.tile([P, 1], F32, tag="r3", name=f"lsum_{nt}")
                nc.scalar.activation(lsum[:], ssum[:], AF.Ln)
                o = sm_p.tile([P, OUT_REAL], F32, tag="o", name=f"o_{nt}")
                nc.vector.tensor_scalar_sub(o[:], z2[:], lsum[:])
                _dma(nc)(out=out_d[nt * P:(nt + 1) * P, :], in_=o[:])

    nc.compile()
    return nc


# ---------------------------------------------------------------- host prep
def host_prep(cfg: Cfg, h, edge_index, W0, b0, Wh, bh, Wo, bo):
    """Build per-core input maps."""
    bf = ml_dtypes.bfloat16
    N, C = cfg.N, cfg.C
    nlr, nloc = cfg.NLOC_REAL, cfg.NLOC
    src = np.asarray(edge_index[0], np.int64)
    dst = np.asarray(edge_index[1], np.int64)

    deg = np.zeros(N, np.float64)
    np.add.at(deg, dst, 1.0)
    deg += 1.0
    dinv = (deg ** -0.5).astype(np.float32)
    deg32 = deg.astype(np.float32)

    # padded global src index: core r, local i -> r*nloc + i
    def pad_idx(g):
        return (g // nlr) * nloc + (g % nlr)

    psrc = pad_idx(src)

    in_maps = []
    for c in range(C):
        lo, hi = c * nlr, min((c + 1) * nlr, N)
        nl = hi - lo
        sel = (dst >= lo) & (dst < hi)
        s_c = psrc[sel]
        d_c = (dst[sel] - lo).astype(np.int64)

        AT = np.zeros((cfg.NFULL, nloc), np.float32)
        np.add.at(AT, (s_c, d_c), 1.0)

        cw = dinv[src[sel]] * dinv[dst[sel]]
        AwT = np.zeros((cfg.NFULL, nloc), np.float32)
        np.add.at(AwT, (s_c, d_c), cw.astype(np.float64).astype(np.float32))
        # self loop 1/deg on the (padded) diagonal
        gids = np.arange(lo, hi)
        AwT[pad_idx(gids), gids - lo] += 1.0 / deg32[gids]

        hT = np.zeros((cfg.IN, nloc), np.float32)
        hT[:, :nl] = np.asarray(h[lo:hi], np.float32).T

        nb = cfg.KT_U * cfg.L + 1
        biases = np.zeros((128, nb), np.float32)
        for l in range(cfg.L):
            b = np.asarray(b0 if l == 0 else bh[l - 1], np.float32)
            for ft in range(cfg.KT_U):
                biases[:, cfg.KT_U * l + ft] = b[ft * 128:(ft + 1) * 128]
        bo_arr = np.asarray(bo, np.float32)
        biases[:len(bo_arr), cfg.KT_U * cfg.L] = bo_arr

        Wo_pad = np.zeros((cfg.U, cfg.OUTP), np.float32)
        Wo_pad[:, :np.asarray(Wo).shape[1]] = np.asarray(Wo, np.float32)

        in_maps.append({
            "hT": hT.reshape(cfg.KT_IN, 128, nloc).copy(),
            "AT": AT.astype(bf).reshape(cfg.KT, 128, nloc).copy(),
            "AwT": AwT.astype(bf).reshape(cfg.KT, 128, nloc).copy(),
            "W0": np.asarray(W0, np.float32).reshape(cfg.KT_IN, 128, cfg.U).copy(),
            "Wh": np.asarray(Wh, np.float32).reshape(cfg.L - 1, cfg.KT_U, 128, cfg.U).copy(),
            "Wo": Wo_pad.reshape(cfg.KT_U, 128, cfg.OUTP).copy(),
            "biases": biases,
        })
    return in_maps


_CACHE = {}


def _get_nc():
    if "nc" not in _CACHE:
        _CACHE["nc"] = build_nc(REAL)
    return _CACHE["nc"]


def kernel(h, edge_index, W0, b0, Wh, bh, Wo, bo, _trace=False, _trace_kwargs=None):
    cfg = REAL
    nc = _get_nc()
    in_maps = host_prep(cfg, h, edge_index, W0, b0, Wh, bh, Wo, bo)
    res = run_bass_kernel_spmd(
        nc, in_maps, list(range(cfg.C)),
        trace=_trace, **(_trace_kwargs or {}),
    )
    outs = [np.asarray(res.results[c]["out"])[:cfg.NLOC_REAL] for c in range(cfg.C)]
    full = np.concatenate(outs, axis=0)[:cfg.N].astype(np.float32)
    if _trace:
        return full, res
    return full



# revision 7
# speedup vs baseline: 2.4894x; 2.4894x over previous
"""JKNet-Maxpool GNN kernel for 8 Trainium2 NeuronCores.

Strategy (graph/data parallel, dense-adjacency aggregation, fp8 PE):
  - Shard dst nodes 8 ways (1250/core, padded to 1280 = 10 tiles of 128).
  - segment_sum over edges == A @ m with A[dst, src] the edge-count matrix.
    A counts are small ints -> exact in fp8 e4m3.  The whole A^T (paired
    layout, fp8) stays RESIDENT in SBUF (13.1 MB) for all 6 hidden layers.
  - Aggregation runs as fp8 DoubleRow matmuls (2 contraction rows/cycle):
    stationary = m pair-chunks [128, 2, 128] fp8, moving = A^T pair rows
    [128, 2, <=512 dst], fp32 PSUM accumulation over 40 src pairs.
  - Activations renormalized per layer by r=1/16 (JK growth ~ x21/layer) so
    messages stay O(1) for fp8; biases are pre-scaled on host, Wo absorbs
    the final descale.  ReLU applies (scale=r, bias) on the scalar engine.
  - Per layer: local GEMM (bf16) -> cast fp8 -> AllGather m (fp8, 2.6 MB)
    -> DoubleRow aggregation -> ReLU/JK-max -> next layer.
  - Final GCN layer reuses the SAME resident A via
    Aw = diag(dinv) A diag(dinv) + diag(1/deg): mo rows pre-scaled by
    dinv, A cast fp8->bf16 on the fly (DVE) for a bf16 aggregation, then
    per-dst dinv scaling + self term + bias after the transpose,
    log_softmax at the end.  No Aw matrix, no extra HBM traffic.
"""

import numpy as np
import ml_dtypes

import concourse.bass as bass
import concourse.bacc as bacc
import concourse.mybir as mybir
import concourse.tile as tile
from concourse.bass_utils import run_bass_kernel_spmd
from concourse.masks import make_identity

BF16 = mybir.dt.bfloat16
F32 = mybir.dt.float32
F8 = mybir.dt.float8e4
AF = mybir.ActivationFunctionType
ALU = mybir.AluOpType
AX = mybir.AxisListType
DR = mybir.MatmulPerfMode.DoubleRow

R = 1.0 / 16.0  # per-layer renormalization factor


# ---------------------------------------------------------------- config
class Cfg:
    def __init__(self, n_nodes, in_feats, units, out_feats, n_layers, n_cores=8):
        self.P = 128
        self.C = n_cores
        self.N = n_nodes
        self.IN = in_feats            # multiple of 128
        self.U = units                # multiple of 128
        self.OUTP = 128               # padded out feats (real out <= 128)
        self.L = n_layers             # hidden GCN layers
        nloc_real = (n_nodes + n_cores - 1) // n_cores
        self.NLOC_REAL = nloc_real
        self.NT_LOC = (nloc_real + 127) // 128
        self.NLOC = self.NT_LOC * 128            # padded local nodes
        self.KT = self.C * self.NT_LOC           # src tiles over padded space
        self.NP = self.KT // 2                   # global src tile pairs
        self.NP_LOC = self.NT_LOC // 2
        self.NFULL = self.KT * 128
        self.KT_IN = in_feats // 128
        self.KT_U = units // 128
        # moving-dim slices for aggregation matmuls (<=512 each)
        self.SLICES = []
        off = 0
        while off < self.NLOC:
            w = min(512, self.NLOC - off)
            self.SLICES.append((off, w))
            off += w


REAL = Cfg(n_nodes=10000, in_feats=512, units=256, out_feats=64, n_layers=6)
OUT_REAL = 64


# ---------------------------------------------------------------- program

DMA_ENGINE = "sync"  # "sync" (HWDGE) or "gpsimd" (SWDGE)


def _dma(nc):
    return (nc.gpsimd if DMA_ENGINE == "gpsimd" else nc.sync).dma_start


def build_nc(cfg: Cfg) -> bass.Bass:
    nc = bacc.Bacc("TRN2", target_bir_lowering=False, num_devices=cfg.C)
    P, L, U = cfg.P, cfg.L, cfg.U

    # ---- dram I/O (per-core contents supplied via in_maps)
    hT_d = nc.dram_tensor("hT", [cfg.KT_IN, P, cfg.NLOC], BF16, kind="ExternalInput")
    ATp_d = nc.dram_tensor("ATp", [cfg.NP, P, 2, cfg.NLOC], F8, kind="ExternalInput")
    W0_d = nc.dram_tensor("W0", [cfg.KT_IN, P, U], BF16, kind="ExternalInput")
    Wh_d = nc.dram_tensor("Wh", [L - 1, cfg.KT_U, P, U], BF16, kind="ExternalInput")
    Wo_d = nc.dram_tensor("Wo", [cfg.KT_U, P, cfg.OUTP], BF16, kind="ExternalInput")
    # packed per-partition biases: col l*KT_U+ft = r^(l+1) * b_l feat tile ft
    bias_d = nc.dram_tensor("biases", [P, cfg.KT_U * L], F32, kind="ExternalInput")
    bo_d = nc.dram_tensor("bo_bc", [P, OUT_REAL], F32, kind="ExternalInput")
    # per-local-node column stats: [:, :, 0]=dinv, [:, :, 1]=1/deg
    cs_d = nc.dram_tensor("colstats", [P, cfg.NT_LOC, 2], F32, kind="ExternalInput")
    out_d = nc.dram_tensor("out", [cfg.NLOC, OUT_REAL], F32, kind="ExternalOutput")

    with tile.TileContext(nc) as tc:
        with (
            tc.tile_pool(name="const", bufs=1) as const_p,
            tc.tile_pool(name="wpool", bufs=1) as w_p,
            tc.tile_pool(name="atres", bufs=1) as atres_p,
            tc.tile_pool(name="xT", bufs=cfg.KT_IN + cfg.KT_U + 2) as x_p,
            tc.tile_pool(name="jk", bufs=1) as jk_p,
            tc.tile_pool(name="mfull", bufs=cfg.NP) as mf_p,
            tc.tile_pool(name="aggf", bufs=1) as aggf_p,
            tc.tile_pool(name="atbf", bufs=4) as atbf_p,
            tc.tile_pool(name="mloc", bufs=4) as ml_p,
            tc.tile_pool(name="small", bufs=6) as sm_p,
            tc.tile_pool(name="moself", bufs=1) as ms_p,
            tc.tile_pool(name="psmm", bufs=2, space="PSUM") as psmm_p,
            tc.tile_pool(name="psagg", bufs=2, space="PSUM") as psagg_p,
            tc.tile_pool(name="dram", bufs=1, space="DRAM") as dram_p,
        ):
            # ---- constants
            biases = const_p.tile([P, cfg.KT_U * L], F32, name="biases_sb")
            _dma(nc)(out=biases[:], in_=bias_d[:])
            bo_bc = const_p.tile([P, OUT_REAL], F32, name="bo_sb")
            _dma(nc)(out=bo_bc[:], in_=bo_d[:])
            cstat = const_p.tile([P, cfg.NT_LOC, 2], F32, name="cs_sb")
            _dma(nc)(out=cstat[:], in_=cs_d[:])
            ident = const_p.tile([P, P], F32, name="ident")
            make_identity(nc, ident[:])

            # ---- x^T tiles (layer 0 = h^T) -- load before AT so layer-0
            # GEMM starts immediately
            xT = []
            for k in range(cfg.KT_IN):
                t = x_p.tile([P, cfg.NLOC], BF16, tag="xT", name=f"xt0_{k}")
                _dma(nc)(out=t[:], in_=hT_d[k])
                xT.append(t)

            # ---- weights resident in SBUF
            w0_sb = []
            for k in range(cfg.KT_IN):
                t = w_p.tile([P, U], BF16, name=f"w0_{k}")
                _dma(nc)(out=t[:], in_=W0_d[k])
                w0_sb.append(t)

            # ---- resident paired adjacency (fp8), streamed in once
            at_res = []
            for kp in range(cfg.NP):
                t = atres_p.tile([P, 2, cfg.NLOC], F8, name=f"at_{kp}")
                _dma(nc)(out=t[:], in_=ATp_d[kp])
                at_res.append(t)

            wh_sb = []
            for l in range(L - 1):
                row = []
                for k in range(cfg.KT_U):
                    t = w_p.tile([P, U], BF16, name=f"wh_{l}_{k}")
                    _dma(nc)(out=t[:], in_=Wh_d[l, k])
                    row.append(t)
                wh_sb.append(row)
            wo_sb = []
            for k in range(cfg.KT_U):
                t = w_p.tile([P, cfg.OUTP], BF16, name=f"wo_{k}")
                _dma(nc)(out=t[:], in_=Wo_d[k])
                wo_sb.append(t)

            # ---- JK running max tiles (normalized domain, bf16)
            jk = [
                jk_p.tile([P, cfg.NLOC], BF16, name=f"jk_{ft}")
                for ft in range(cfg.KT_U)
            ]

            # ---- collective bounce buffers
            m_loc_d = dram_p.tile([cfg.NP_LOC, P, 2, U], F8, name="m_loc_d")
            m_full_ds = [
                dram_p.tile([cfg.NP, P, 2, U], F8, name=f"m_full_d{l}",
                            addr_space="Shared")
                for l in range(L)
            ]
            mo_loc_d = dram_p.tile([cfg.NP_LOC, P, 2, cfg.OUTP], BF16,
                                   name="mo_loc_d")
            mo_full_d = dram_p.tile([cfg.NP, P, 2, cfg.OUTP], BF16,
                                    name="mo_full_d", addr_space="Shared")

            # ================= hidden layers =================
            for l in range(L):
                xt_in = xT
                w_tiles = w0_sb if l == 0 else wh_sb[l - 1]
                kt = len(xt_in)
                # --- local GEMM (bf16) -> fp8 m tiles -> paired DRAM layout
                for nt in range(cfg.NT_LOC):
                    ps = psmm_p.tile([P, U], F32, tag="mm", name=f"ps_l{l}_{nt}")
                    for k in range(kt):
                        nc.tensor.matmul(
                            ps[:],
                            lhsT=xt_in[k][:, nt * P:(nt + 1) * P],
                            rhs=w_tiles[k][:],
                            start=(k == 0),
                            stop=(k == kt - 1),
                        )
                    mt = ml_p.tile([P, U], F8, tag="mloc", name=f"m_l{l}_{nt}")
                    nc.vector.tensor_copy(out=mt[:], in_=ps[:])
                    _dma(nc)(out=m_loc_d[nt // 2, :, nt % 2, :], in_=mt[:])
                nc.gpsimd.collective_compute(
                    "AllGather",
                    ALU.bypass,
                    replica_groups=[list(range(cfg.C))],
                    ins=[m_loc_d.opt()],
                    outs=[m_full_ds[l].opt()],
                )
                m_sb = []
                for kp in range(cfg.NP):
                    t = mf_p.tile([P, 2, U], F8, tag="mfull", name=f"mf_l{l}_{kp}")
                    _dma(nc)(out=t[:], in_=m_full_ds[l][kp])
                    m_sb.append(t)
                # --- fp8 DoubleRow aggregation over all 40 src pairs
                ps_list = [
                    psagg_p.tile([P, cfg.NLOC], F32, tag="agg",
                                 name=f"agg_l{l}_{ft}")
                    for ft in range(cfg.KT_U)
                ]
                for kp in range(cfg.NP):
                    for ft in range(cfg.KT_U):
                        for off, w in cfg.SLICES:
                            nc.tensor.matmul(
                                ps_list[ft][:, off:off + w],
                                lhsT=m_sb[kp][:, :, ft * P:(ft + 1) * P],
                                rhs=at_res[kp][:, :, off:off + w],
                                start=(kp == 0),
                                stop=(kp == cfg.NP - 1),
                                perf_mode=DR,
                            )
                # --- ReLU (scale=r, pre-scaled bias) -> bf16 x~; JK max
                xT = []
                for ft in range(cfg.KT_U):
                    xt_new = x_p.tile([P, cfg.NLOC], BF16, tag="xT",
                                      name=f"xt{l + 1}_{ft}")
                    nc.scalar.activation(
                        xt_new[:], ps_list[ft][:], AF.Relu,
                        bias=biases[:, cfg.KT_U * l + ft:cfg.KT_U * l + ft + 1],
                        scale=R,
                    )
                    xT.append(xt_new)
                    if l == 0:
                        nc.vector.tensor_copy(out=jk[ft][:], in_=xt_new[:])
                    else:
                        nc.vector.tensor_scalar_mul(jk[ft][:], jk[ft][:], R)
                        nc.vector.tensor_tensor(
                            out=jk[ft][:], in0=jk[ft][:], in1=xt_new[:],
                            op=ALU.max,
                        )

            # ================= final layer =================
            # mo = jk~ @ Wo' (Wo' = Wo/r^L on host); keep raw mo for the
            # self term, AllGather dinv-scaled bf16 rows for the messages.
            mo_self = []
            for nt in range(cfg.NT_LOC):
                ps = psmm_p.tile([P, cfg.OUTP], F32, tag="mm", name=f"ps_f_{nt}")
                for k in range(cfg.KT_U):
                    nc.tensor.matmul(
                        ps[:],
                        lhsT=jk[k][:, nt * P:(nt + 1) * P],
                        rhs=wo_sb[k][:],
                        start=(k == 0),
                        stop=(k == cfg.KT_U - 1),
                    )
                mself = ms_p.tile([P, cfg.OUTP], F32, name=f"mself_{nt}")
                nc.scalar.copy(out=mself[:], in_=ps[:])
                mo_self.append(mself)
                mdt = ml_p.tile([P, cfg.OUTP], BF16, tag="mloc", name=f"md_{nt}")
                nc.vector.tensor_scalar_mul(
                    mdt[:], ps[:], cstat[:, nt, 0:1])
                _dma(nc)(out=mo_loc_d[nt // 2, :, nt % 2, :], in_=mdt[:])
            nc.gpsimd.collective_compute(
                "AllGather",
                ALU.bypass,
                replica_groups=[list(range(cfg.C))],
                ins=[mo_loc_d.opt()],
                outs=[mo_full_d.opt()],
            )
            mo_sb = []
            for kp in range(cfg.NP):
                t = mf_p.tile([P, 2, cfg.OUTP], BF16, tag="mfull",
                              name=f"mo_{kp}")
                _dma(nc)(out=t[:], in_=mo_full_d[kp])
                mo_sb.append(t)
            # --- bf16 aggregation: moving = fp8 A^T cast to bf16 on DVE
            psF = psagg_p.tile([P, cfg.NLOC], F32, tag="agg", name="agg_fin")
            for kp in range(cfg.NP):
                ab = atbf_p.tile([P, 2, cfg.NLOC], BF16, tag="atbf",
                                 name=f"ab_{kp}")
                nc.vector.tensor_copy(out=ab[:], in_=at_res[kp][:])
                for i in range(2):
                    for off, w in cfg.SLICES:
                        nc.tensor.matmul(
                            psF[:, off:off + w],
                            lhsT=mo_sb[kp][:, i, :],
                            rhs=ab[:, i, off:off + w],
                            start=(kp == 0 and i == 0),
                            stop=(kp == cfg.NP - 1 and i == 1),
                        )
            aggF = aggf_p.tile([P, cfg.NLOC], F32, name="aggF")
            nc.scalar.copy(out=aggF[:], in_=psF[:])
            # --- per-dst fixups (after transpose) + log_softmax
            for nt in range(cfg.NT_LOC):
                ps_t = psmm_p.tile([P, P], F32, tag="mm", name=f"pst_{nt}")
                nc.tensor.transpose(
                    out=ps_t[:], in_=aggF[:, nt * P:(nt + 1) * P],
                    identity=ident[:],
                )
                z = sm_p.tile([P, OUT_REAL], F32, tag="z", name=f"z_{nt}")
                # z = dinv[dst]*aggF^T + (1/deg)*mo_self + bo
                nc.vector.tensor_scalar_mul(
                    z[:], ps_t[:, 0:OUT_REAL], cstat[:, nt, 0:1])
                z2 = sm_p.tile([P, OUT_REAL], F32, tag="z2", name=f"z2_{nt}")
                nc.vector.tensor_scalar_mul(
                    z2[:], mo_self[nt][:, 0:OUT_REAL], cstat[:, nt, 1:2])
                nc.vector.tensor_tensor(out=z[:], in0=z[:], in1=z2[:],
                                        op=ALU.add)
                nc.vector.tensor_tensor(out=z[:], in0=z[:], in1=bo_bc[:],
                                        op=ALU.add)
                rmax = sm_p.tile([P, 1], F32, tag="r1", name=f"rmax_{nt}")
                nc.vector.reduce_max(rmax[:], z[:], axis=AX.X)
                nc.vector.tensor_scalar_sub(z2[:], z[:], rmax[:])
                ez = sm_p.tile([P, OUT_REAL], F32, tag="ez", name=f"ez_{nt}")
                nc.scalar.activation(ez[:], z2[:], AF.Exp)
                ssum = sm_p.tile([P, 1], F32, tag="r2", name=f"ssum_{nt}")
                nc.vector.reduce_sum(ssum[:], ez[:], axis=AX.X)
                lsum = sm_p.tile([P, 1], F32, tag="r3", name=f"lsum_{nt}")
                nc.scalar.activation(lsum[:], ssum[:], AF.Ln)
                o = sm_p.tile([P, OUT_REAL], F32, tag="o", name=f"o_{nt}")
                nc.vector.tensor_scalar_sub(o[:], z2[:], lsum[:])
                _dma(nc)(out=out_d[nt * P:(nt + 1) * P, :], in_=o[:])

    nc.compile()
    return nc


# ---------------------------------------------------------------- host prep
def host_prep(cfg: Cfg, h, edge_index, W0, b0, Wh, bh, Wo, bo):
    """Build per-core input maps."""
    bf = ml_dtypes.bfloat16
    f8 = ml_dtypes.float8_e4m3
    N, C = cfg.N, cfg.C
    nlr, nloc = cfg.NLOC_REAL, cfg.NLOC
    src = np.asarray(edge_index[0], np.int64)
    dst = np.asarray(edge_index[1], np.int64)

    deg = np.zeros(N, np.float64)
    np.add.at(deg, dst, 1.0)
    deg += 1.0
    dinv = (deg ** -0.5).astype(np.float32)
    invdeg = (1.0 / deg).astype(np.float32)

    # padded global src index: core r, local i -> r*nloc + i
    def pad_idx(g):
        return (g // nlr) * nloc + (g % nlr)

    psrc = pad_idx(src)

    # shared (replicated) tensors
    W0_b = np.asarray(W0, np.float32).reshape(cfg.KT_IN, 128, cfg.U)
    Wh_b = np.asarray(Wh, np.float32).reshape(cfg.L - 1, cfg.KT_U, 128, cfg.U)
    Wo_pad = np.zeros((cfg.U, cfg.OUTP), np.float32)
    Wo_pad[:, :np.asarray(Wo).shape[1]] = np.asarray(Wo, np.float32) / (R ** cfg.L)
    Wo_b = Wo_pad.reshape(cfg.KT_U, 128, cfg.OUTP)

    biases = np.zeros((128, cfg.KT_U * cfg.L), np.float32)
    for l in range(cfg.L):
        b = np.asarray(b0 if l == 0 else bh[l - 1], np.float32) * (R ** (l + 1))
        for ft in range(cfg.KT_U):
            biases[:, cfg.KT_U * l + ft] = b[ft * 128:(ft + 1) * 128]
    bo_bc = np.broadcast_to(
        np.asarray(bo, np.float32)[None, :], (128, OUT_REAL)).copy()

    in_maps = []
    for c in range(C):
        lo, hi = c * nlr, min((c + 1) * nlr, N)
        nl = hi - lo
        sel = (dst >= lo) & (dst < hi)
        s_c = psrc[sel]
        d_c = (dst[sel] - lo).astype(np.int64)

        AT = np.zeros((cfg.NFULL, nloc), np.float32)
        np.add.at(AT, (s_c, d_c), 1.0)
        # paired layout [NP, 128, 2, NLOC]
        ATp = AT.reshape(cfg.NP, 2, 128, nloc).transpose(0, 2, 1, 3)

        hT = np.zeros((cfg.IN, nloc), np.float32)
        hT[:, :nl] = np.asarray(h[lo:hi], np.float32).T

        # partition p, tile nt <-> local node nt*128+p
        cs2 = np.zeros((128, cfg.NT_LOC, 2), np.float32)
        gids = np.arange(lo, hi)
        li = gids - lo
        cs2[li % 128, li // 128, 0] = dinv[gids]
        cs2[li % 128, li // 128, 1] = invdeg[gids]

        in_maps.append({
            "hT": hT.astype(bf).reshape(cfg.KT_IN, 128, nloc).copy(),
            "ATp": ATp.astype(f8).copy(),
            "W0": W0_b.astype(bf).copy(),
            "Wh": Wh_b.astype(bf).copy(),
            "Wo": Wo_b.astype(bf).copy(),
            "biases": biases,
            "bo_bc": bo_bc,
            "colstats": cs2,
        })
    return in_maps


_CACHE = {}


def _get_nc():
    if "nc" not in _CACHE:
        _CACHE["nc"] = build_nc(REAL)
    return _CACHE["nc"]


def kernel(h, edge_index, W0, b0, Wh, bh, Wo, bo, _trace=False, _trace_kwargs=None):
    cfg = REAL
    nc = _get_nc()
    in_maps = host_prep(cfg, h, edge_index, W0, b0, Wh, bh, Wo, bo)
    res = run_bass_kernel_spmd(
        nc, in_maps, list(range(cfg.C)),
        trace=_trace, **(_trace_kwargs or {}),
    )
    outs = [np.asarray(res.results[c]["out"])[:cfg.NLOC_REAL] for c in range(cfg.C)]
    full = np.concatenate(outs, axis=0)[:cfg.N].astype(np.float32)
    if _trace:
        return full, res
    return full
